# revision 15
# baseline (speedup 1.0000x reference)
"""Trainium2 Bass kernel for nn_MultiHeadFusionAttention (B=4, S=1024, DM=1024, H=16, DK=DV=64).

Sharding: 8 cores = (batch b = core//2) x (head-half hh = core%2, 8 heads each).
Each core computes QKV projections for its 8 heads, gated+masked softmax
attention, the attn output slice, the selective-attention AV product, and a
partial fc projection (over its 512 hv rows).  Core pairs ReduceScatter-add
their fc partials, then each core applies struct_gate, residual, and LayerNorm
on its half of the tokens.

All matmuls run as float32r (full PE rate at N>=512, ~1.6e-4 rel err).
"""

import numpy as np

import concourse.bacc as bacc
import concourse.mybir as mybir
import concourse.tile as tile
from concourse import bass_utils
from concourse.masks import make_identity

dt = mybir.dt

Alu = mybir.AluOpType
Act = mybir.ActivationFunctionType

B, S, DM, H, DK, DV = 4, 1024, 1024, 16, 64, 64
NH = 8          # heads per core
HV = NH * DV    # 512 hv rows per core
SH = S // 2     # tokens per core after reduce-scatter
N_CORES = 8
INV_TEMP = 1.0 / 8.0
LN_EPS = 1e-5
NEG_BIG = -30000.0
P = 128

f32, f32r, u8 = dt.float32, dt.float32r, dt.uint8


def _build_nc(with_cc=True, stages="ABC"):
    nc = bacc.Bacc(num_devices=N_CORES)

    Q = nc.dram_tensor("q_in", [S, DM], f32, kind="ExternalInput")
    K = nc.dram_tensor("k_in", [S, DM], f32, kind="ExternalInput")
    V = nc.dram_tensor("v_in", [S, DM], f32, kind="ExternalInput")
    KG = nc.dram_tensor("kg_in", [NH, S, S], f32, kind="ExternalInput")
    MASK = nc.dram_tensor("mask_in", [S, S], u8, kind="ExternalInput")
    KRES = nc.dram_tensor("kres_in", [SH, DM], f32, kind="ExternalInput")
    SG = nc.dram_tensor("sg_in", [SH, 1], f32, kind="ExternalInput")
    WQ = nc.dram_tensor("wq_in", [DM, HV], f32, kind="ExternalInput")
    WK = nc.dram_tensor("wk_in", [DM, HV], f32, kind="ExternalInput")
    WV = nc.dram_tensor("wv_in", [DM, HV], f32, kind="ExternalInput")
    BQ = nc.dram_tensor("bq_in", [HV, 1], f32, kind="ExternalInput")
    BK = nc.dram_tensor("bk_in", [HV, 1], f32, kind="ExternalInput")
    BVR = nc.dram_tensor("bvr_in", [P, HV], f32, kind="ExternalInput")
    FCW = nc.dram_tensor("fcw_in", [HV, DM], f32, kind="ExternalInput")
    FCBR = nc.dram_tensor("fcbr_in", [P, DM], f32, kind="ExternalInput")
    GR = nc.dram_tensor("gr_in", [P, DM], f32, kind="ExternalInput")
    BR = nc.dram_tensor("br_in", [P, DM], f32, kind="ExternalInput")

    ATTN = nc.dram_tensor("attn_out", [NH, S, S], f32, kind="ExternalOutput")
    OUT = nc.dram_tensor("out_out", [SH, DM], f32, kind="ExternalOutput")

    with tile.TileContext(nc) as tc:
        with tc.tile_pool(name="pBC", bufs=1) as pBC:
            # outT stacked per head: fcT[h] = (attn_h^T @ vh_h)^T  -> [DV, S]
            fcT = [pBC.tile([DV, S], f32r, tag=f"fcT{h}", name=f"fcT{h}") for h in range(NH)]

            with tc.tile_pool(name="pAB", bufs=1) as pAB:
                qhT = [pAB.tile([DK, S], f32r, tag=f"qhT{h}", name=f"qhT{h}") for h in range(NH)]
                khT = [pAB.tile([DK, S], f32r, tag=f"khT{h}", name=f"khT{h}") for h in range(NH)]
                vh = [pAB.tile([P, HV], f32r, tag=f"vh{si}", name=f"vh{si}") for si in range(S // P)]
                bq_t = pAB.tile([DK, NH], f32, tag="bq_t")  # [64, 8]: one column per head
                bk_t = pAB.tile([DK, NH], f32, tag="bk_t")

                # ---------------- stage A: transposes + projections ----------------
                with tc.tile_pool(name="pA", bufs=2) as pA, \
                     tc.tile_pool(name="psT", bufs=2, space="PSUM") as psT, \
                     tc.tile_pool(name="psP", bufs=2, space="PSUM") as psP:
                    ident = pA.tile([P, P], f32, tag="ident", bufs=1)
                    make_identity(nc, ident)

                    # biases: bq/bk as [128, 4] so each head's 64 values sit on
                    # partitions (h%2)*64..  loaded column-per-head-pair.
                    nc.sync.dma_start(bq_t[:], BQ.rearrange("(h p) one -> p (h one)", p=DK))
                    nc.sync.dma_start(bk_t[:], BK.rearrange("(h p) one -> p (h one)", p=DK))
                    bv_rep = pAB.tile([P, HV], f32, tag="bv_rep")
                    nc.sync.dma_start(bv_rep[:], BVR[:])

                    xT = [pA.tile([P, S], f32r, tag=f"xT{di}", name=f"xT{di}", bufs=1) for di in range(DM // P)]

                    def transpose_in(x_dram, wname):
                        # load x [S, DM], produce xT[di] [128, S] (fp32r-rounded)
                        for si in range(S // P):
                            xs = pA.tile([P, DM], f32, tag="x_in")
                            nc.sync.dma_start(xs[:], x_dram[si * P:(si + 1) * P, :])
                            for di in range(DM // P):
                                pt = psT.tile([P, P], f32, tag="pt")
                                nc.tensor.transpose(pt[:], xs[:, di * P:(di + 1) * P], ident[:])
                                nc.scalar.copy(xT[di][:, si * P:(si + 1) * P], pt[:])
                        # rounded weight slice [DM, HV] as 8 x [128, HV]
                        wr = []
                        for di in range(DM // P):
                            w_in = pA.tile([P, HV], f32, tag="w_in")
                            nc.sync.dma_start(w_in[:], wname[di * P:(di + 1) * P, :])
                            wt_ = pA.tile([P, HV], f32r, tag=f"w_r{di}", bufs=1)
                            nc.scalar.copy(wt_[:], w_in[:])
                            wr.append(wt_)
                        return wr

                    # --- Q ---
                    wq_r = transpose_in(Q, WQ)
                    for h in range(NH):
                        for sc in range(2):
                            pq = psP.tile([DK, 512], f32, tag="pq")
                            for di in range(DM // P):
                                nc.tensor.matmul(
                                    pq[:], wq_r[di][:, h * DK:(h + 1) * DK],
                                    xT[di][:, sc * 512:(sc + 1) * 512],
                                    start=(di == 0), stop=(di == DM // P - 1))
                            nc.scalar.activation(
                                qhT[h][:, sc * 512:(sc + 1) * 512], pq[:], Act.Identity,
                                bias=bq_t[:, h:h + 1])
                    # --- K ---
                    wk_r = transpose_in(K, WK)
                    for h in range(NH):
                        for sc in range(2):
                            pk = psP.tile([DK, 512], f32, tag="pq")
                            for di in range(DM // P):
                                nc.tensor.matmul(
                                    pk[:], wk_r[di][:, h * DK:(h + 1) * DK],
                                    xT[di][:, sc * 512:(sc + 1) * 512],
                                    start=(di == 0), stop=(di == DM // P - 1))
                            nc.scalar.activation(
                                khT[h][:, sc * 512:(sc + 1) * 512], pk[:], Act.Identity,
                                bias=bk_t[:, h:h + 1])
                    # --- V ---  vh[si] [128 s, HV] = v @ wv + bv
                    wv_r = transpose_in(V, WV)
                    for si in range(S // P):
                        pv = psP.tile([P, HV], f32, tag="pv")
                        for di in range(DM // P):
                            nc.tensor.matmul(
                                pv[:], xT[di][:, si * P:(si + 1) * P], wv_r[di][:],
                                start=(di == 0), stop=(di == DM // P - 1))
                        nc.vector.tensor_add(vh[si][:], pv[:], bv_rep[:])

                # ---------------- stage B: attention ----------------
                with tc.tile_pool(name="pB", bufs=2) as pB, \
                     tc.tile_pool(name="pBkg", bufs=4) as pBkg, \
                     tc.tile_pool(name="psS", bufs=2, space="PSUM") as psS, \
                     tc.tile_pool(name="psAV", bufs=2, space="PSUM") as psAV:
                    negbig = pB.tile([P, 1], f32, tag="negbig", bufs=1)
                    nc.any.memset(negbig[:], NEG_BIG)
                    mask_t = [pB.tile([P, S], u8, tag=f"mask{qi}", name=f"mask{qi}", bufs=1) for qi in range(S // P)]
                    for qi in range(S // P):
                        nc.sync.dma_start(mask_t[qi][:], MASK[qi * P:(qi + 1) * P, :])

                    for h in range(NH if "B" in stages else 0):
                        av = psAV.tile([DV, S], f32, tag="av")
                        for qi in range(S // P):
                            kgt = pBkg.tile([P, S], f32, tag="kg")
                            nc.sync.dma_start(kgt[:], KG[h, qi * P:(qi + 1) * P, :])
                            sps = psS.tile([P, S], f32, tag="sps")
                            for kc in range(2):
                                nc.tensor.matmul(
                                    sps[:, kc * 512:(kc + 1) * 512],
                                    qhT[h][:, qi * P:(qi + 1) * P],
                                    khT[h][:, kc * 512:(kc + 1) * 512],
                                    start=True, stop=True)
                            wt = pB.tile([P, S], f32, tag="wt")
                            nc.vector.scalar_tensor_tensor(
                                wt[:], sps[:], INV_TEMP, kgt[:], Alu.mult, Alu.mult)
                            nc.vector.copy_predicated(
                                wt[:], mask_t[qi][:], negbig[:].broadcast_to([P, S]))
                            et = pB.tile([P, S], f32, tag="et")
                            zr = pB.tile([P, 1], f32, tag="zr")
                            nc.scalar.activation(et[:], wt[:], Act.Exp, accum_out=zr[:])
                            rt = pB.tile([P, 1], f32, tag="rt")
                            nc.vector.reciprocal(rt[:], zr[:])
                            at = pB.tile([P, S], f32r, tag="at")
                            nc.scalar.activation(at[:], et[:], Act.Copy, scale=rt[:])
                            nc.sync.dma_start(ATTN[h, qi * P:(qi + 1) * P, :],
                                              at[:].bitcast(f32))
                            for kc in range(2):
                                nc.tensor.matmul(
                                    av[:, kc * 512:(kc + 1) * 512],
                                    vh[qi][:, h * DV:(h + 1) * DV],
                                    at[:, kc * 512:(kc + 1) * 512],
                                    start=(qi == 0), stop=(qi == S // P - 1),
                                    skip_group_check=True)
                        nc.scalar.copy(fcT[h][:], av[:])

            # ---------------- stage C: fc + ReduceScatter + LN ----------------
            with tc.tile_pool(name="pC", bufs=2) as pC, \
                 tc.tile_pool(name="psC", bufs=2, space="PSUM") as psC, \
                 tc.tile_pool(name="dramC", bufs=1, space="DRAM") as dramC:
                fcw_r = []
                for i in range(NH if ("C" in stages or "F" in stages) else 0):
                    fw = pC.tile([DV, DM], f32, tag="fw_in")
                    nc.sync.dma_start(fw[:], FCW[i * DV:(i + 1) * DV, :])
                    fr = pC.tile([DV, DM], f32r, tag=f"fcw{i}", bufs=1)
                    nc.scalar.copy(fr[:], fw[:])
                    fcw_r.append(fr)

                fc_bounce = dramC.tile([S, DM], f32)
                rs_out = dramC.tile([SH, DM], f32)
                for si in range((S // P) if ("C" in stages or "F" in stages) else 0):
                    pf = psC.tile([P, DM], f32, tag="pf")
                    for dc in range(2):
                        for hc in range(NH):
                            nc.tensor.matmul(
                                pf[:, dc * 512:(dc + 1) * 512],
                                fcT[hc][:, si * P:(si + 1) * P],
                                fcw_r[hc][:, dc * 512:(dc + 1) * 512],
                                start=(hc == 0), stop=(hc == NH - 1),
                                skip_group_check=True)
                    fo = pC.tile([P, DM], f32, tag="fo")
                    nc.scalar.copy(fo[:], pf[:])
                    nc.sync.dma_start(fc_bounce[si * P:(si + 1) * P, :], fo[:])

                if with_cc and ("C" in stages or "R" in stages):
                    nc.gpsimd.collective_compute(
                        "ReduceScatter", Alu.add,
                        replica_groups=[[0, 1], [2, 3], [4, 5], [6, 7]],
                        ins=[fc_bounce[:]], outs=[rs_out[:]])
                elif "C" in stages or "R" in stages:
                    # debug: pretend partner partial is zero
                    for si in range(SH // P):
                        dbg = pC.tile([P, DM], f32, tag="xt")
                        nc.sync.dma_start(dbg[:], fc_bounce[si * P:(si + 1) * P, :])
                        nc.sync.dma_start(rs_out[si * P:(si + 1) * P, :], dbg[:])

                if "C" not in stages and "L" not in stages:
                    dummy = pC.tile([P, 1], f32, tag="eps_t", bufs=1)
                    nc.any.memset(dummy[:], 0.0)
                    nc.sync.dma_start(OUT[0:P, 0:1], dummy[:])
                eps_t = pC.tile([P, 1], f32, tag="eps_t2", bufs=1)
                nc.any.memset(eps_t[:], LN_EPS)
                fcb_t = pC.tile([P, DM], f32, tag="fcb_t", bufs=1)
                g_t = pC.tile([P, DM], f32, tag="g_t", bufs=1)
                b_t = pC.tile([P, DM], f32, tag="b_t", bufs=1)
                nc.sync.dma_start(fcb_t[:], FCBR[:])
                nc.sync.dma_start(g_t[:], GR[:])
                nc.sync.dma_start(b_t[:], BR[:])

                for si in range((SH // P) if ("C" in stages or "L" in stages) else 0):
                    xt = pC.tile([P, DM], f32, tag="xt")
                    nc.sync.dma_start(xt[:], rs_out[si * P:(si + 1) * P, :])
                    kt = pC.tile([P, DM], f32, tag="kt")
                    nc.sync.dma_start(kt[:], KRES[si * P:(si + 1) * P, :])
                    sgt = pC.tile([P, 1], f32, tag="sgt")
                    nc.sync.dma_start(sgt[:], SG[si * P:(si + 1) * P, :])

                    # x = (fc_partial_sum + fc_b) * sg + residual
                    nc.vector.tensor_add(xt[:], xt[:], fcb_t[:])
                    x2 = pC.tile([P, DM], f32, tag="x2")
                    nc.vector.scalar_tensor_tensor(
                        x2[:], xt[:], sgt[:], kt[:], Alu.mult, Alu.add)
                    # LayerNorm over free dim
                    s1 = pC.tile([P, 1], f32, tag="s1")
                    nc.vector.reduce_sum(s1[:], x2[:], axis=mybir.AxisListType.X)
                    mu = pC.tile([P, 1], f32, tag="mu")
                    nc.vector.tensor_scalar_mul(mu[:], s1[:], 1.0 / DM)
                    nc.vector.tensor_scalar(x2[:], x2[:], mu[:], None, Alu.subtract)
                    sq = pC.tile([P, DM], f32, tag="sq")
                    s2 = pC.tile([P, 1], f32, tag="s2")
                    nc.scalar.activation(sq[:], x2[:], Act.Square, accum_out=s2[:])
                    sd = pC.tile([P, 1], f32, tag="sd")
                    nc.scalar.activation(sd[:], s2[:], Act.Sqrt, bias=eps_t[:],
                                         scale=1.0 / DM)
                    rstd = pC.tile([P, 1], f32, tag="rstd")
                    nc.vector.reciprocal(rstd[:], sd[:])
                    y = pC.tile([P, DM], f32, tag="y")
                    nc.vector.scalar_tensor_tensor(
                        y[:], x2[:], rstd[:], g_t[:], Alu.mult, Alu.mult)
                    nc.vector.tensor_add(y[:], y[:], b_t[:])
                    nc.sync.dma_start(OUT[si * P:(si + 1) * P, :], y[:])

    nc.finalize()
    return nc


_NC_CACHE = None


def _get_nc():
    global _NC_CACHE
    if _NC_CACHE is None:
        _NC_CACHE = _build_nc()
    return _NC_CACHE


def make_in_maps(q, k, v, k_gate, struct_gate, mask, wq, bq, wk, bk, wv, bv,
                 fc_w, fc_b, ln_g, ln_b):
    q = np.asarray(q, np.float32)
    k = np.asarray(k, np.float32)
    v = np.asarray(v, np.float32)
    k_gate = np.asarray(k_gate, np.float32)
    struct_gate = np.asarray(struct_gate, np.float32)
    mask_u8 = np.asarray(mask).astype(np.uint8)
    wq = np.asarray(wq, np.float32); bq = np.asarray(bq, np.float32)
    wk = np.asarray(wk, np.float32); bk = np.asarray(bk, np.float32)
    wv = np.asarray(wv, np.float32); bv = np.asarray(bv, np.float32)
    fc_w = np.asarray(fc_w, np.float32); fc_b = np.asarray(fc_b, np.float32)
    ln_g = np.asarray(ln_g, np.float32); ln_b = np.asarray(ln_b, np.float32)

    in_maps = []
    for c in range(N_CORES):
        b, hh = c // 2, c % 2
        h0 = hh * NH
        cols = slice(h0 * DK, h0 * DK + HV)
        in_maps.append({
            "q_in": np.ascontiguousarray(q[b]),
            "k_in": np.ascontiguousarray(k[b]),
            "v_in": np.ascontiguousarray(v[b]),
            "kg_in": np.ascontiguousarray(k_gate[b, h0:h0 + NH]),
            "mask_in": np.ascontiguousarray(mask_u8[b]),
            "kres_in": np.ascontiguousarray(k[b, hh * SH:(hh + 1) * SH]),
            "sg_in": np.ascontiguousarray(
                struct_gate[b, hh * SH:(hh + 1) * SH].reshape(SH, 1)),
            "wq_in": np.ascontiguousarray(wq[:, cols]),
            "wk_in": np.ascontiguousarray(wk[:, cols]),
            "wv_in": np.ascontiguousarray(wv[:, cols]),
            "bq_in": np.ascontiguousarray(bq[cols].reshape(HV, 1)),
            "bk_in": np.ascontiguousarray(bk[cols].reshape(HV, 1)),
            "bvr_in": np.ascontiguousarray(
                np.broadcast_to(bv[cols], (P, HV)).copy()),
            "fcw_in": np.ascontiguousarray(fc_w[h0 * DV:h0 * DV + HV]),
            "fcbr_in": np.ascontiguousarray(np.broadcast_to(fc_b, (P, DM)).copy()),
            "gr_in": np.ascontiguousarray(np.broadcast_to(ln_g, (P, DM)).copy()),
            "br_in": np.ascontiguousarray(np.broadcast_to(ln_b, (P, DM)).copy()),
        })
    return in_maps


def gather_outputs(results):
    out = np.empty((B, S, DM), np.float32)
    attn = np.empty((B, H, S, S), np.float32)
    for c in range(N_CORES):
        b, hh = c // 2, c % 2
        h0 = hh * NH
        out[b, hh * SH:(hh + 1) * SH] = results[c]["out_out"]
        attn[b, h0:h0 + NH] = results[c]["attn_out"]
    return out, attn


def kernel(q, k, v, k_gate, struct_gate, mask, wq, bq, wk, bk, wv, bv,
           fc_w, fc_b, ln_g, ln_b):
    nc = _get_nc()
    in_maps = make_in_maps(q, k, v, k_gate, struct_gate, mask, wq, bq, wk, bk,
                           wv, bv, fc_w, fc_b, ln_g, ln_b)
    res = bass_utils.run_bass_kernel_spmd(nc, in_maps, core_ids=list(range(N_CORES)))
    return gather_outputs(res.results)
